# revision 13
# baseline (speedup 1.0000x reference)
"""Trainium2 Bass kernel for the additive-attention problem (V3.6).

reference math:
    rec[b,h]    = sum_r rnn_state[b,r] * W_rec[h,r]
    scores[t,b] = sum_h tanh(enc[t,b,h] + rec[b,h]) * w_score[h] + b_score + mask[t,b]
    out         = softmax(scores, axis=t)          # (T, B) float32

Sharding: data-parallel over B across 8 cores (BL=4 batch columns per core).
Softmax is over T (core-local) -> no collectives.

Layout: h on partitions (host ships encT = enc.transpose(1,2,0) as fp16,
halving HBM traffic vs f32; ~47us DMA floor at ~358GB/s/core).  Per (b, hc)
tile [p=h%128, f=t (4096)]:
  - DMA fp16 tile (1MB, 8KB contiguous rows)
  - ACT: Y = tanh(enc + rec) in ONE op - rec[b, hc*128+p] is a per-partition
    bias AP (the broadcast add costs nothing on VectorE)
  - PE: score reduction over h: lhsT = Y[:, tc*128:(tc+1)*128] (stationary,
    fp16), rhs = w_score column for hc -> psum col [t%128, 1]; per-column
    atomic start+stop groups, 4 hc column-sets combined on VectorE (chained,
    single PSUM operand per op, mask folded into the chain).
Scores land as [p=t%128, f=(tc,b)]: exp, PE transpose, row sums (bf16),
block-mask matmul (bf16 ldweights) broadcasts per-b totals, reciprocal,
scale, DMA out as (BL,T) contiguous rows.  b_score cancels in softmax; no
max-subtraction needed (|scores| <~ 25).  fp16 on the elementwise path.

ACT is the bottleneck (16 x (4096+222)cyc @ 1.2GHz ~= 58us back-to-back,
measured gap=0).  Critical-path engineering:
  - dummy tanh on the identity tile hoists ACT_TABLE_LOAD to t~=0
  - NO artificial PE warmup: a burst of heavy PE work drops the other
    engines' clocks ~20% for the whole kernel (shared DVFS budget)
  - all small tensors are pre-arranged on host to their exact SBUF layout
    (contiguous descriptors; a strided mask DMA cost ~2us of SWDGE time)
  - small loads interleaved with the first enc tile halves on the sync ring
    (the DMA/HBM pipe is slow for the first ~15us regardless of ring usage;
    fancier multi-ring arrangements measured the same or worse)
  - m4 + mask ride the GpSimd SWDGE ring
  - last tile split in 2 so the final PE pass overlaps the tanh drain
"""

import numpy as np

T, B, H, R = 4096, 32, 512, 512
NCORES = 8
BL = B // NCORES          # 4 local batch columns
HC = H // 128             # 4 h-chunks
RC = R // 128             # 4 r-chunks
NTC = T // 128            # 32 t-chunks of 128

_GRAPH = None


def _build_graph():
    import concourse.bass as bass
    import concourse.tile as tile
    from concourse import bacc, mybir
    from concourse.masks import make_identity

    f32 = mybir.dt.float32
    f16 = mybir.dt.float16
    bf16 = mybir.dt.bfloat16
    nc = bacc.Bacc()

    encT = nc.declare_dram_parameter("encT", [BL, H, T], f16, isOutput=False)
    maskd = nc.declare_dram_parameter("maskd", [128, NTC, BL], f32, isOutput=False)
    rnnd = nc.declare_dram_parameter("rnnd", [128, RC, BL], f16, isOutput=False)
    wrecd = nc.declare_dram_parameter(
        "wrecd", [HC, 128, RC, 128], f16, isOutput=False
    )
    wcold = nc.declare_dram_parameter("wcold", [128, HC], f16, isOutput=False)
    m4d = nc.declare_dram_parameter("m4", [128, 128], bf16, isOutput=False)
    out = nc.declare_dram_parameter("out", [BL, T], f32, isOutput=True)

    with tile.TileContext(nc) as tc:
        with (
            tc.tile_pool(name="singles", bufs=1) as singles,
            tc.tile_pool(name="xpool", bufs=6) as xpool,
            tc.tile_pool(name="ypool", bufs=6) as ypool,
            tc.tile_pool(name="spool", bufs=2, space="PSUM") as spool,
            tc.tile_pool(name="spool1", bufs=1, space="PSUM") as spool1,
        ):
            ident = singles.tile([128, 128], f32)
            make_identity(nc, ident[:])
            # dummy tanh: forces ACT_TABLE_LOAD at t~=0 (no DMA dependency)
            dummy = singles.tile([128, 1], f32)
            nc.scalar.activation(
                out=dummy[:], in_=ident[:, 0:1],
                func=mybir.ActivationFunctionType.Tanh,
            )

            encv = encT.rearrange("b (hc p) t -> b hc p t", p=128)
            H2 = T // 2

            # sync/HWDGE ring: rec-chain inputs interleaved with the first
            # enc tile halves (each issue costs ~650ns of engine time)
            rnn_sb = singles.tile([128, RC, BL], f16)
            nc.sync.dma_start(out=rnn_sb[:], in_=rnnd[:])
            wrec_sb = singles.tile([128, HC, RC, 128], f16)
            nc.sync.dma_start(out=wrec_sb[:, 0], in_=wrecd[0])
            X00 = xpool.tile([128, T], f16)
            nc.sync.dma_start(out=X00[:, :H2], in_=encv[0, 0, :, :H2])
            nc.sync.dma_start(out=wrec_sb[:, 1], in_=wrecd[1])
            nc.sync.dma_start(out=X00[:, H2:], in_=encv[0, 0, :, H2:])
            nc.sync.dma_start(out=wrec_sb[:, 2], in_=wrecd[2])
            nc.sync.dma_start(out=wrec_sb[:, 3], in_=wrecd[3])
            wcol = singles.tile([128, HC], f16)
            nc.sync.dma_start(out=wcol[:], in_=wcold[:])
            # gpsimd SWDGE ring: only needed for the softmax tail
            m4 = singles.tile([128, 128], bf16)
            nc.gpsimd.dma_start(out=m4[:], in_=m4d[:])
            mask_sb = singles.tile([128, NTC, BL], f32)
            nc.gpsimd.dma_start(out=mask_sb[:], in_=maskd[:])

            # rec.T[h, b] = sum_r W_rec[h, r] * rnn[b, r], per h-chunk
            rec_sb = singles.tile([128, HC, BL], f32)
            for hc in range(HC):
                rp = spool.tile([128, BL], f32, tag="rec")
                for rc in range(RC):
                    nc.tensor.matmul(
                        rp[:],
                        lhsT=wrec_sb[:, hc, rc],
                        rhs=rnn_sb[:, rc, :],
                        start=(rc == 0),
                        stop=(rc == RC - 1),
                    )
                nc.vector.tensor_copy(out=rec_sb[:, hc, :], in_=rp[:])

            scores = singles.tile([128, NTC, BL], f32)

            def tile_work(b, hc, Pp, X, nsp, emit_dma):
                Y = ypool.tile([128, T], f16)
                for s in range(nsp):
                    sl = slice(s * T // nsp, (s + 1) * T // nsp)
                    if emit_dma:
                        nc.sync.dma_start(out=X[:, sl], in_=encv[b, hc, :, sl])
                    nc.scalar.activation(
                        out=Y[:, sl],
                        in_=X[:, sl],
                        func=mybir.ActivationFunctionType.Tanh,
                        bias=rec_sb[:, hc, b : b + 1],
                    )
                    for tcng in range(s * NTC // nsp, (s + 1) * NTC // nsp):
                        c = hc * NTC + tcng
                        nc.tensor.matmul(
                            Pp[:, c : c + 1],
                            lhsT=Y[:, tcng * 128 : (tcng + 1) * 128],
                            rhs=wcol[:, hc : hc + 1],
                            start=True,
                            stop=True,
                        )

            # ---------- main loop: per (b, hc) tile [128, T] ----------
            for b in range(BL):
                Pp = spool.tile([128, HC * NTC], f32, tag="partials")
                for hc in range(HC):
                    if b == 0 and hc == 0:
                        tile_work(b, hc, Pp, X00, 2, emit_dma=False)
                        continue
                    last = b == BL - 1 and hc == HC - 1
                    X = xpool.tile([128, T], f16)
                    tile_work(b, hc, Pp, X, 2 if last else 1, emit_dma=True)
                # combine the 4 hc partials + mask -> scores[:, :, b]
                # (never two PSUM operands in one DVE op - single PSUM rd port)
                acc = ypool.tile([128, NTC], f32, tag="comb")
                nc.vector.tensor_copy(out=acc[:], in_=Pp[:, 0:NTC])
                nc.vector.tensor_add(
                    out=acc[:], in0=acc[:], in1=Pp[:, NTC : 2 * NTC]
                )
                nc.vector.tensor_add(
                    out=acc[:], in0=acc[:], in1=mask_sb[:, :, b]
                )
                nc.vector.tensor_add(
                    out=acc[:], in0=acc[:], in1=Pp[:, 2 * NTC : 3 * NTC]
                )
                nc.vector.tensor_add(
                    out=scores[:, :, b],
                    in0=acc[:],
                    in1=Pp[:, 3 * NTC : 4 * NTC],
                )

            # ---------- exp, softmax normalization, output ----------
            E = singles.tile([128, 128], f32)
            nc.scalar.activation(
                out=E[:], in_=scores[:].rearrange("p tc b -> p (tc b)"),
                func=mybir.ActivationFunctionType.Exp,
            )
            # transpose: (p=t%128, f=(tc,b)) -> (p=(tc,b), f=t%128)
            attT = spool1.tile([128, 128], f32, tag="attT")
            nc.tensor.transpose(out=attT[:], in_=E[:], identity=ident[:])
            row_sums = singles.tile([128, 1], bf16)
            # bf16 row sums: the per-b errors average out in the f32 matmul
            # accumulate (~7e-4 rel on denom); buys a 1-pass bf16 LDWEIGHTS
            with nc.allow_low_precision(reason="bf16 rowsums, f32 accum"):
                nc.vector.tensor_reduce(
                    out=row_sums[:], in_=attT[:], axis=mybir.AxisListType.X,
                    op=mybir.AluOpType.add,
                )
            denom = spool1.tile([128, 1], f32, tag="denom")
            nc.tensor.matmul(
                denom[:], lhsT=m4[:], rhs=row_sums[:], start=True, stop=True
            )
            recip = singles.tile([128, 1], f32)
            nc.vector.reciprocal(out=recip[:], in_=denom[:])
            att_out = singles.tile([128, 128], f32)
            nc.vector.tensor_scalar_mul(
                out=att_out[:], in0=attT[:], scalar1=recip[:]
            )
            # partition p = (tc, b) holds 128 contiguous t values for col b
            nc.sync.dma_start(
                out=out.rearrange("b (tc tp) -> tc b tp", tp=128),
                in_=att_out[:],
            )

    nc.compile()
    return nc


def _get_graph():
    global _GRAPH
    if _GRAPH is None:
        _GRAPH = _build_graph()
    return _GRAPH


def make_in_maps(enc, mask, rnn_state, W_rec, w_score):
    import ml_dtypes

    enc16 = np.asarray(enc, dtype=np.float16)
    # [T, B, H] -> [B, H, T]
    encT_full = np.ascontiguousarray(enc16.transpose(1, 2, 0))
    # [HC, 128, RC, 128]: wrecd[hc, p, rc, w] = W_rec[hc*128+w, rc*128+p]
    wrecd = np.ascontiguousarray(
        W_rec.T.astype(np.float16)
        .reshape(RC, 128, HC, 128)
        .transpose(2, 1, 0, 3)
    )
    wcol = np.ascontiguousarray(
        np.asarray(w_score, dtype=np.float16).reshape(HC, 128).T
    )
    m4 = (np.arange(128)[:, None] % BL == np.arange(128)[None, :] % BL)
    m4 = np.ascontiguousarray(m4.astype(ml_dtypes.bfloat16))
    in_maps = []
    for c in range(NCORES):
        sl = slice(c * BL, (c + 1) * BL)
        # rnnd[p, rc, b] = rnn_state[c*BL+b, rc*128+p]
        rnnd = np.ascontiguousarray(
            rnn_state[sl].astype(np.float16).T.reshape(RC, 128, BL)
            .transpose(1, 0, 2)
        )
        # maskd[p, tc, b] = mask[tc*128+p, c*BL+b]
        maskc = np.ascontiguousarray(
            np.asarray(mask[:, sl], dtype=np.float32).reshape(NTC, 128, BL)
            .transpose(1, 0, 2)
        )
        in_maps.append(
            {
                "encT": np.ascontiguousarray(encT_full[sl]),
                "maskd": maskc,
                "rnnd": rnnd,
                "wrecd": wrecd,
                "wcold": wcol,
                "m4": m4,
            }
        )
    return in_maps


def kernel(
    encoded_contribution,
    mask,
    rnn_state,
    prev_att_weights,
    W_rec,
    w_score,
    b_score,
):
    from concourse.bass_utils import run_bass_kernel_spmd

    nc = _get_graph()
    in_maps = make_in_maps(
        np.asarray(encoded_contribution),
        np.asarray(mask),
        np.asarray(rnn_state),
        np.asarray(W_rec),
        np.asarray(w_score),
    )
    res = run_bass_kernel_spmd(nc, in_maps, list(range(NCORES)))
    outs = [np.asarray(res.results[c]["out"]) for c in range(NCORES)]
    return np.concatenate([o.T for o in outs], axis=1).astype(np.float32)


# revision 16
# speedup vs baseline: 1.0029x; 1.0029x over previous
"""Trainium2 Bass kernel for the additive-attention problem (V3.8).

reference math:
    rec[b,h]    = sum_r rnn_state[b,r] * W_rec[h,r]
    scores[t,b] = sum_h tanh(enc[t,b,h] + rec[b,h]) * w_score[h] + b_score + mask[t,b]
    out         = softmax(scores, axis=t)          # (T, B) float32

Sharding: data-parallel over B across 8 cores (BL=4 batch columns per core).
Softmax is over T (core-local) -> no collectives.

Layout: h on partitions (host ships encT = enc.transpose(1,2,0) as fp16,
halving HBM traffic vs f32; ~47us DMA floor at ~358GB/s/core).  Per (b, hc)
tile [p=h%128, f=t (4096)]:
  - DMA fp16 tile (1MB, 8KB contiguous rows)
  - ACT: Y = tanh(enc + rec) in ONE op - rec[b, hc*128+p] is a per-partition
    bias AP (the broadcast add costs nothing on VectorE)
  - PE: score reduction over h: lhsT = Y[:, tc*128:(tc+1)*128] (stationary,
    fp16), rhs = w_score column for hc -> psum col [t%128, 1]; per-column
    atomic start+stop groups, 4 hc column-sets combined on VectorE (chained,
    single PSUM operand per op, mask folded into the chain).
Scores land as [p=t%128, f=(tc,b)]: exp, PE transpose, row sums (bf16),
block-mask matmul (bf16 ldweights) broadcasts per-b totals, reciprocal,
scale, DMA out as (BL,T) contiguous rows.  b_score cancels in softmax; no
max-subtraction needed (|scores| <~ 25).  fp16 on the elementwise path.

ACT is the bottleneck (16 x (4096+222)cyc @ 1.2GHz ~= 58us back-to-back,
measured gap=0).  Critical-path engineering:
  - dummy tanh on the identity tile hoists ACT_TABLE_LOAD to t~=0
  - NO artificial PE warmup: a burst of heavy PE work drops the other
    engines' clocks ~20% for the whole kernel (shared DVFS budget)
  - all small tensors are pre-arranged on host to their exact SBUF layout
    (contiguous descriptors; a strided mask DMA cost ~2us of SWDGE time)
  - small loads interleaved with the first enc tile halves on the sync ring
    (the DMA/HBM pipe is slow for the first ~15us regardless of ring usage;
    fancier multi-ring arrangements measured the same or worse)
  - m4 + mask ride the GpSimd SWDGE ring
  - last tile split in 2 so the final PE pass overlaps the tanh drain
"""

import numpy as np

T, B, H, R = 4096, 32, 512, 512
NCORES = 8
BL = B // NCORES          # 4 local batch columns
HC = H // 128             # 4 h-chunks
RC = R // 128             # 4 r-chunks
NTC = T // 128            # 32 t-chunks of 128

_GRAPH = None


def _build_graph():
    import concourse.bass as bass
    import concourse.tile as tile
    from concourse import bacc, mybir
    from concourse.masks import make_identity

    f32 = mybir.dt.float32
    f16 = mybir.dt.float16
    bf16 = mybir.dt.bfloat16
    nc = bacc.Bacc()

    encT = nc.declare_dram_parameter("encT", [BL, H, T], f16, isOutput=False)
    maskd = nc.declare_dram_parameter("maskd", [128, NTC, BL], f32, isOutput=False)
    rnnd = nc.declare_dram_parameter("rnnd", [128, RC, BL], f16, isOutput=False)
    wrecd = nc.declare_dram_parameter(
        "wrecd", [HC, 128, RC, 128], f16, isOutput=False
    )
    wcold = nc.declare_dram_parameter("wcold", [128, HC], f16, isOutput=False)
    m4d = nc.declare_dram_parameter("m4", [128, 128], bf16, isOutput=False)
    out = nc.declare_dram_parameter("out", [BL, T], f32, isOutput=True)

    with tile.TileContext(nc) as tc:
        with (
            tc.tile_pool(name="singles", bufs=1) as singles,
            tc.tile_pool(name="xpool", bufs=6) as xpool,
            tc.tile_pool(name="ypool", bufs=6) as ypool,
            tc.tile_pool(name="spool", bufs=2, space="PSUM") as spool,
            tc.tile_pool(name="spool1", bufs=1, space="PSUM") as spool1,
        ):
            ident = singles.tile([128, 128], f32)
            make_identity(nc, ident[:])
            # dummy tanh: forces ACT_TABLE_LOAD at t~=0 (no DMA dependency)
            dummy = singles.tile([128, 1], f32)
            nc.scalar.activation(
                out=dummy[:], in_=ident[:, 0:1],
                func=mybir.ActivationFunctionType.Tanh,
            )

            encv = encT.rearrange("b (hc p) t -> b hc p t", p=128)
            H2 = T // 2

            # sync/HWDGE ring: rec-chain inputs interleaved with the first
            # enc tile halves (each issue costs ~650ns of engine time)
            rnn_sb = singles.tile([128, RC, BL], f16)
            nc.sync.dma_start(out=rnn_sb[:], in_=rnnd[:])
            wrec_sb = singles.tile([128, HC, RC, 128], f16)
            nc.sync.dma_start(out=wrec_sb[:, 0], in_=wrecd[0])
            X00 = xpool.tile([128, T], f16)
            nc.sync.dma_start(out=X00[:, :H2], in_=encv[0, 0, :, :H2])
            nc.sync.dma_start(out=wrec_sb[:, 1], in_=wrecd[1])
            nc.sync.dma_start(out=X00[:, H2:], in_=encv[0, 0, :, H2:])
            nc.sync.dma_start(out=wrec_sb[:, 2], in_=wrecd[2])
            nc.sync.dma_start(out=wrec_sb[:, 3], in_=wrecd[3])
            wcol = singles.tile([128, HC], f16)
            nc.sync.dma_start(out=wcol[:], in_=wcold[:])
            # tile (0,1): first half rides the otherwise-idle scalar HWDGE
            # ring (transfers in parallel with the sync ring during the slow
            # early-DMA window), second half follows on sync after wcol
            X01 = xpool.tile([128, T], f16)
            nc.scalar.dma_start(out=X01[:, :H2], in_=encv[0, 1, :, :H2])
            nc.sync.dma_start(out=X01[:, H2:], in_=encv[0, 1, :, H2:])
            # gpsimd SWDGE ring: only needed for the softmax tail
            m4 = singles.tile([128, 128], bf16)
            nc.gpsimd.dma_start(out=m4[:], in_=m4d[:])
            mask_sb = singles.tile([128, NTC, BL], f32)
            nc.gpsimd.dma_start(out=mask_sb[:], in_=maskd[:])

            # rec.T[h, b] = sum_r W_rec[h, r] * rnn[b, r], per h-chunk
            rec_sb = singles.tile([128, HC, BL], f32)
            for hc in range(HC):
                rp = spool.tile([128, BL], f32, tag="rec")
                for rc in range(RC):
                    nc.tensor.matmul(
                        rp[:],
                        lhsT=wrec_sb[:, hc, rc],
                        rhs=rnn_sb[:, rc, :],
                        start=(rc == 0),
                        stop=(rc == RC - 1),
                    )
                nc.vector.tensor_copy(out=rec_sb[:, hc, :], in_=rp[:])

            scores = singles.tile([128, NTC, BL], f32)

            def tile_work(b, hc, Pp, X, nsp, emit_dma):
                Y = ypool.tile([128, T], f16)
                for s in range(nsp):
                    sl = slice(s * T // nsp, (s + 1) * T // nsp)
                    if emit_dma:
                        nc.sync.dma_start(out=X[:, sl], in_=encv[b, hc, :, sl])
                    nc.scalar.activation(
                        out=Y[:, sl],
                        in_=X[:, sl],
                        func=mybir.ActivationFunctionType.Tanh,
                        bias=rec_sb[:, hc, b : b + 1],
                    )
                    for tcng in range(s * NTC // nsp, (s + 1) * NTC // nsp):
                        c = hc * NTC + tcng
                        nc.tensor.matmul(
                            Pp[:, c : c + 1],
                            lhsT=Y[:, tcng * 128 : (tcng + 1) * 128],
                            rhs=wcol[:, hc : hc + 1],
                            start=True,
                            stop=True,
                        )

            # ---------- main loop: per (b, hc) tile [128, T] ----------
            for b in range(BL):
                Pp = spool.tile([128, HC * NTC], f32, tag="partials")
                for hc in range(HC):
                    if b == 0 and hc in (0, 1):
                        tile_work(b, hc, Pp, X00 if hc == 0 else X01, 2,
                                  emit_dma=False)
                        continue
                    last = b == BL - 1 and hc == HC - 1
                    X = xpool.tile([128, T], f16)
                    tile_work(b, hc, Pp, X, 2 if last else 1, emit_dma=True)
                # combine the 4 hc partials + mask -> scores[:, :, b]
                # (never two PSUM operands in one DVE op - single PSUM rd port)
                acc = ypool.tile([128, NTC], f32, tag="comb")
                nc.vector.tensor_copy(out=acc[:], in_=Pp[:, 0:NTC])
                nc.vector.tensor_add(
                    out=acc[:], in0=acc[:], in1=Pp[:, NTC : 2 * NTC]
                )
                nc.vector.tensor_add(
                    out=acc[:], in0=acc[:], in1=mask_sb[:, :, b]
                )
                nc.vector.tensor_add(
                    out=acc[:], in0=acc[:], in1=Pp[:, 2 * NTC : 3 * NTC]
                )
                nc.vector.tensor_add(
                    out=scores[:, :, b],
                    in0=acc[:],
                    in1=Pp[:, 3 * NTC : 4 * NTC],
                )

            # ---------- exp, softmax normalization, output ----------
            E = singles.tile([128, 128], f32)
            nc.scalar.activation(
                out=E[:], in_=scores[:].rearrange("p tc b -> p (tc b)"),
                func=mybir.ActivationFunctionType.Exp,
            )
            # transpose: (p=t%128, f=(tc,b)) -> (p=(tc,b), f=t%128)
            attT = spool1.tile([128, 128], f32, tag="attT")
            nc.tensor.transpose(out=attT[:], in_=E[:], identity=ident[:])
            row_sums = singles.tile([128, 1], bf16)
            # bf16 row sums: the per-b errors average out in the f32 matmul
            # accumulate (~7e-4 rel on denom); buys a 1-pass bf16 LDWEIGHTS
            with nc.allow_low_precision(reason="bf16 rowsums, f32 accum"):
                nc.vector.tensor_reduce(
                    out=row_sums[:], in_=attT[:], axis=mybir.AxisListType.X,
                    op=mybir.AluOpType.add,
                )
            denom = spool1.tile([128, 1], f32, tag="denom")
            nc.tensor.matmul(
                denom[:], lhsT=m4[:], rhs=row_sums[:], start=True, stop=True
            )
            recip = singles.tile([128, 1], f32)
            nc.vector.reciprocal(out=recip[:], in_=denom[:])
            att_out = singles.tile([128, 128], f32)
            nc.vector.tensor_scalar_mul(
                out=att_out[:], in0=attT[:], scalar1=recip[:]
            )
            # partition p = (tc, b) holds 128 contiguous t values for col b
            nc.sync.dma_start(
                out=out.rearrange("b (tc tp) -> tc b tp", tp=128),
                in_=att_out[:],
            )

    nc.compile()
    return nc


def _get_graph():
    global _GRAPH
    if _GRAPH is None:
        _GRAPH = _build_graph()
    return _GRAPH


def make_in_maps(enc, mask, rnn_state, W_rec, w_score):
    import ml_dtypes

    enc16 = np.asarray(enc, dtype=np.float16)
    # [T, B, H] -> [B, H, T]
    encT_full = np.ascontiguousarray(enc16.transpose(1, 2, 0))
    # [HC, 128, RC, 128]: wrecd[hc, p, rc, w] = W_rec[hc*128+w, rc*128+p]
    wrecd = np.ascontiguousarray(
        W_rec.T.astype(np.float16)
        .reshape(RC, 128, HC, 128)
        .transpose(2, 1, 0, 3)
    )
    wcol = np.ascontiguousarray(
        np.asarray(w_score, dtype=np.float16).reshape(HC, 128).T
    )
    m4 = (np.arange(128)[:, None] % BL == np.arange(128)[None, :] % BL)
    m4 = np.ascontiguousarray(m4.astype(ml_dtypes.bfloat16))
    in_maps = []
    for c in range(NCORES):
        sl = slice(c * BL, (c + 1) * BL)
        # rnnd[p, rc, b] = rnn_state[c*BL+b, rc*128+p]
        rnnd = np.ascontiguousarray(
            rnn_state[sl].astype(np.float16).T.reshape(RC, 128, BL)
            .transpose(1, 0, 2)
        )
        # maskd[p, tc, b] = mask[tc*128+p, c*BL+b]
        maskc = np.ascontiguousarray(
            np.asarray(mask[:, sl], dtype=np.float32).reshape(NTC, 128, BL)
            .transpose(1, 0, 2)
        )
        in_maps.append(
            {
                "encT": np.ascontiguousarray(encT_full[sl]),
                "maskd": maskc,
                "rnnd": rnnd,
                "wrecd": wrecd,
                "wcold": wcol,
                "m4": m4,
            }
        )
    return in_maps


def kernel(
    encoded_contribution,
    mask,
    rnn_state,
    prev_att_weights,
    W_rec,
    w_score,
    b_score,
):
    from concourse.bass_utils import run_bass_kernel_spmd

    nc = _get_graph()
    in_maps = make_in_maps(
        np.asarray(encoded_contribution),
        np.asarray(mask),
        np.asarray(rnn_state),
        np.asarray(W_rec),
        np.asarray(w_score),
    )
    res = run_bass_kernel_spmd(nc, in_maps, list(range(NCORES)))
    outs = [np.asarray(res.results[c]["out"]) for c in range(NCORES)]
    return np.concatenate([o.T for o in outs], axis=1).astype(np.float32)


# revision 18
# speedup vs baseline: 1.0043x; 1.0013x over previous
"""Trainium2 Bass kernel for the additive-attention problem (V3.8).

reference math:
    rec[b,h]    = sum_r rnn_state[b,r] * W_rec[h,r]
    scores[t,b] = sum_h tanh(enc[t,b,h] + rec[b,h]) * w_score[h] + b_score + mask[t,b]
    out         = softmax(scores, axis=t)          # (T, B) float32

Sharding: data-parallel over B across 8 cores (BL=4 batch columns per core).
Softmax is over T (core-local) -> no collectives.

Layout: h on partitions (host ships encT = enc.transpose(1,2,0) as fp16,
halving HBM traffic vs f32; ~47us DMA floor at ~358GB/s/core).  Per (b, hc)
tile [p=h%128, f=t (4096)]:
  - DMA fp16 tile (1MB, 8KB contiguous rows)
  - ACT: Y = tanh(enc + rec) in ONE op - rec[b, hc*128+p] is a per-partition
    bias AP (the broadcast add costs nothing on VectorE)
  - PE: score reduction over h: lhsT = Y[:, tc*128:(tc+1)*128] (stationary,
    fp16), rhs = w_score column for hc -> psum col [t%128, 1]; per-column
    atomic start+stop groups, 4 hc column-sets combined on VectorE (chained,
    single PSUM operand per op, mask folded into the chain).
Scores land as [p=t%128, f=(tc,b)]: exp, PE transpose, row sums (bf16),
block-mask matmul (bf16 ldweights) broadcasts per-b totals, reciprocal,
scale, DMA out as (BL,T) contiguous rows.  b_score cancels in softmax; no
max-subtraction needed (|scores| <~ 25).  fp16 on the elementwise path.

ACT is the bottleneck (16 x (4096+222)cyc @ 1.2GHz ~= 58us back-to-back,
measured gap=0).  Critical-path engineering:
  - dummy tanh on the identity tile hoists ACT_TABLE_LOAD to t~=0
  - NO artificial PE warmup: a burst of heavy PE work drops the other
    engines' clocks ~20% for the whole kernel (shared DVFS budget)
  - all small tensors are pre-arranged on host to their exact SBUF layout
    (contiguous descriptors; a strided mask DMA cost ~2us of SWDGE time)
  - small loads interleaved with the first enc tile halves on the sync ring
    (the DMA/HBM pipe is slow for the first ~15us regardless of ring usage;
    fancier multi-ring arrangements measured the same or worse)
  - m4 + mask ride the GpSimd SWDGE ring
  - last tile split in 2 so the final PE pass overlaps the tanh drain
"""

import numpy as np

T, B, H, R = 4096, 32, 512, 512
NCORES = 8
BL = B // NCORES          # 4 local batch columns
HC = H // 128             # 4 h-chunks
RC = R // 128             # 4 r-chunks
NTC = T // 128            # 32 t-chunks of 128

_GRAPH = None


def _build_graph():
    import concourse.bass as bass
    import concourse.tile as tile
    from concourse import bacc, mybir
    from concourse.masks import make_identity

    f32 = mybir.dt.float32
    f16 = mybir.dt.float16
    bf16 = mybir.dt.bfloat16
    nc = bacc.Bacc()

    encT = nc.declare_dram_parameter("encT", [BL, H, T], f16, isOutput=False)
    maskd = nc.declare_dram_parameter("maskd", [128, NTC, BL], f32, isOutput=False)
    rnnd = nc.declare_dram_parameter("rnnd", [128, RC, BL], f16, isOutput=False)
    wrecd = nc.declare_dram_parameter(
        "wrecd", [HC, 128, RC, 128], f16, isOutput=False
    )
    wcold = nc.declare_dram_parameter("wcold", [128, HC], f16, isOutput=False)
    m4d = nc.declare_dram_parameter("m4", [128, 128], bf16, isOutput=False)
    out = nc.declare_dram_parameter("out", [BL, T], f32, isOutput=True)

    with tile.TileContext(nc) as tc:
        with (
            tc.tile_pool(name="singles", bufs=1) as singles,
            tc.tile_pool(name="xpool", bufs=6) as xpool,
            tc.tile_pool(name="ypool", bufs=6) as ypool,
            tc.tile_pool(name="spool", bufs=2, space="PSUM") as spool,
            tc.tile_pool(name="spool1", bufs=1, space="PSUM") as spool1,
        ):
            ident = singles.tile([128, 128], f32)
            make_identity(nc, ident[:])
            # dummy tanh: forces ACT_TABLE_LOAD at t~=0 (no DMA dependency)
            dummy = singles.tile([128, 1], f32)
            nc.scalar.activation(
                out=dummy[:], in_=ident[:, 0:1],
                func=mybir.ActivationFunctionType.Tanh,
            )

            encv = encT.rearrange("b (hc p) t -> b hc p t", p=128)
            H2 = T // 2

            # sync/HWDGE ring: rec-chain inputs interleaved with the first
            # enc tile halves (each issue costs ~650ns of engine time)
            rnn_sb = singles.tile([128, RC, BL], f16)
            nc.sync.dma_start(out=rnn_sb[:], in_=rnnd[:])
            wrec_sb = singles.tile([128, HC, RC, 128], f16)
            nc.sync.dma_start(out=wrec_sb[:, 0], in_=wrecd[0])
            X00 = xpool.tile([128, T], f16)
            nc.sync.dma_start(out=X00[:, :H2], in_=encv[0, 0, :, :H2])
            nc.sync.dma_start(out=wrec_sb[:, 1], in_=wrecd[1])
            nc.sync.dma_start(out=X00[:, H2:], in_=encv[0, 0, :, H2:])
            nc.sync.dma_start(out=wrec_sb[:, 2], in_=wrecd[2])
            nc.sync.dma_start(out=wrec_sb[:, 3], in_=wrecd[3])
            wcol = singles.tile([128, HC], f16)
            nc.sync.dma_start(out=wcol[:], in_=wcold[:])
            # tile (0,1): first half rides the otherwise-idle scalar HWDGE
            # ring (transfers in parallel with the sync ring during the slow
            # early-DMA window), second half follows on sync after wcol
            X01 = xpool.tile([128, T], f16)
            nc.scalar.dma_start(out=X01[:, :H2], in_=encv[0, 1, :, :H2])
            nc.sync.dma_start(out=X01[:, H2:], in_=encv[0, 1, :, H2:])
            # gpsimd SWDGE ring: only needed for the softmax tail
            m4 = singles.tile([128, 128], bf16)
            nc.gpsimd.dma_start(out=m4[:], in_=m4d[:])
            mask_sb = singles.tile([128, NTC, BL], f32)
            nc.gpsimd.dma_start(out=mask_sb[:], in_=maskd[:])

            # rec.T[h, b] = sum_r W_rec[h, r] * rnn[b, r], per h-chunk
            rec_sb = singles.tile([128, HC, BL], f32)
            for hc in range(HC):
                rp = spool.tile([128, BL], f32, tag="rec")
                for rc in range(RC):
                    nc.tensor.matmul(
                        rp[:],
                        lhsT=wrec_sb[:, hc, rc],
                        rhs=rnn_sb[:, rc, :],
                        start=(rc == 0),
                        stop=(rc == RC - 1),
                    )
                nc.vector.tensor_copy(out=rec_sb[:, hc, :], in_=rp[:])

            scores = singles.tile([128, NTC, BL], f32)

            def tile_work(b, hc, Pp, X, nsp, emit_dma):
                Y = ypool.tile([128, T], f16)
                for s in range(nsp):
                    sl = slice(s * T // nsp, (s + 1) * T // nsp)
                    if emit_dma:
                        nc.sync.dma_start(out=X[:, sl], in_=encv[b, hc, :, sl])
                    nc.scalar.activation(
                        out=Y[:, sl],
                        in_=X[:, sl],
                        func=mybir.ActivationFunctionType.Tanh,
                        bias=rec_sb[:, hc, b : b + 1],
                    )
                    for tcng in range(s * NTC // nsp, (s + 1) * NTC // nsp):
                        c = hc * NTC + tcng
                        nc.tensor.matmul(
                            Pp[:, c : c + 1],
                            lhsT=Y[:, tcng * 128 : (tcng + 1) * 128],
                            rhs=wcol[:, hc : hc + 1],
                            start=True,
                            stop=True,
                        )

            # ---------- main loop: per (b, hc) tile [128, T] ----------
            for b in range(BL):
                Pp = spool.tile([128, HC * NTC], f32, tag="partials")
                for hc in range(HC):
                    if b == 0 and hc in (0, 1):
                        tile_work(b, hc, Pp, X00 if hc == 0 else X01, 2,
                                  emit_dma=False)
                        continue
                    last = b == BL - 1 and hc == HC - 1
                    X = xpool.tile([128, T], f16)
                    tile_work(b, hc, Pp, X, 2 if last else 1, emit_dma=True)
                # combine the 4 hc partials + mask -> scores[:, :, b]
                # (never two PSUM operands in one DVE op - single PSUM rd port)
                acc = ypool.tile([128, NTC], f32, tag="comb")
                nc.vector.tensor_copy(out=acc[:], in_=Pp[:, 0:NTC])
                nc.vector.tensor_add(
                    out=acc[:], in0=acc[:], in1=Pp[:, NTC : 2 * NTC]
                )
                nc.vector.tensor_add(
                    out=acc[:], in0=acc[:], in1=mask_sb[:, :, b]
                )
                nc.vector.tensor_add(
                    out=acc[:], in0=acc[:], in1=Pp[:, 2 * NTC : 3 * NTC]
                )
                nc.vector.tensor_add(
                    out=scores[:, :, b],
                    in0=acc[:],
                    in1=Pp[:, 3 * NTC : 4 * NTC],
                )

            # ---------- exp, softmax normalization, output ----------
            E = singles.tile([128, 128], f32)
            nc.scalar.activation(
                out=E[:], in_=scores[:].rearrange("p tc b -> p (tc b)"),
                func=mybir.ActivationFunctionType.Exp,
            )
            # transpose: (p=t%128, f=(tc,b)) -> (p=(tc,b), f=t%128)
            attT = spool1.tile([128, 128], f32, tag="attT")
            nc.tensor.transpose(out=attT[:], in_=E[:], identity=ident[:])
            row_sums = singles.tile([128, 1], bf16)
            # bf16 row sums: the per-b errors average out in the f32 matmul
            # accumulate (~7e-4 rel on denom); buys a 1-pass bf16 LDWEIGHTS
            with nc.allow_low_precision(reason="bf16 rowsums, f32 accum"):
                nc.vector.tensor_reduce(
                    out=row_sums[:], in_=attT[:], axis=mybir.AxisListType.X,
                    op=mybir.AluOpType.add,
                )
            denom = spool1.tile([128, 1], f32, tag="denom")
            nc.tensor.matmul(
                denom[:], lhsT=m4[:], rhs=row_sums[:], start=True, stop=True
            )
            recip = singles.tile([128, 1], f32)
            nc.vector.reciprocal(out=recip[:], in_=denom[:])
            att_out = singles.tile([128, 128], f32)
            nc.vector.tensor_scalar_mul(
                out=att_out[:], in0=attT[:], scalar1=recip[:]
            )
            # partition p = (tc, b) holds 128 contiguous t values for col b
            nc.sync.dma_start(
                out=out.rearrange("b (tc tp) -> tc b tp", tp=128),
                in_=att_out[:],
            )

    nc.compile()
    return nc


def _get_graph():
    global _GRAPH
    if _GRAPH is None:
        _GRAPH = _build_graph()
    return _GRAPH


def make_in_maps(enc, mask, rnn_state, W_rec, w_score):
    import ml_dtypes

    enc16 = np.asarray(enc, dtype=np.float16)
    # [T, B, H] -> [B, H, T]
    encT_full = np.ascontiguousarray(enc16.transpose(1, 2, 0))
    # [HC, 128, RC, 128]: wrecd[hc, p, rc, w] = W_rec[hc*128+w, rc*128+p]
    wrecd = np.ascontiguousarray(
        W_rec.T.astype(np.float16)
        .reshape(RC, 128, HC, 128)
        .transpose(2, 1, 0, 3)
    )
    wcol = np.ascontiguousarray(
        np.asarray(w_score, dtype=np.float16).reshape(HC, 128).T
    )
    m4 = (np.arange(128)[:, None] % BL == np.arange(128)[None, :] % BL)
    m4 = np.ascontiguousarray(m4.astype(ml_dtypes.bfloat16))
    in_maps = []
    for c in range(NCORES):
        sl = slice(c * BL, (c + 1) * BL)
        # rnnd[p, rc, b] = rnn_state[c*BL+b, rc*128+p]
        rnnd = np.ascontiguousarray(
            rnn_state[sl].astype(np.float16).T.reshape(RC, 128, BL)
            .transpose(1, 0, 2)
        )
        # maskd[p, tc, b] = mask[tc*128+p, c*BL+b]
        maskc = np.ascontiguousarray(
            np.asarray(mask[:, sl], dtype=np.float32).reshape(NTC, 128, BL)
            .transpose(1, 0, 2)
        )
        in_maps.append(
            {
                "encT": np.ascontiguousarray(encT_full[sl]),
                "maskd": maskc,
                "rnnd": rnnd,
                "wrecd": wrecd,
                "wcold": wcol,
                "m4": m4,
            }
        )
    return in_maps


def kernel(
    encoded_contribution,
    mask,
    rnn_state,
    prev_att_weights,
    W_rec,
    w_score,
    b_score,
):
    from concourse.bass_utils import run_bass_kernel_spmd

    nc = _get_graph()
    in_maps = make_in_maps(
        np.asarray(encoded_contribution),
        np.asarray(mask),
        np.asarray(rnn_state),
        np.asarray(W_rec),
        np.asarray(w_score),
    )
    res = run_bass_kernel_spmd(nc, in_maps, list(range(NCORES)))
    outs = [np.asarray(res.results[c]["out"]) for c in range(NCORES)]
    return np.concatenate([o.T for o in outs], axis=1).astype(np.float32)
